# revision 51
# baseline (speedup 1.0000x reference)
"""DynamicSegmentationHead Trainium2 kernel (fp16, split-L2, wide-evac).

Data-parallel over the 16 clip-frames: each of the 8 NeuronCores handles 2
frames (100 queries). Per core, the head is a chain of PE matmuls over
"query groups" (15q -> 120 rows):

  controller:  params = hs @ W_ctrl.T + b_ctrl   (fp16 in, fp32 psum)
  L1:  y1 = relu(W0eff @ [feat; -gx; -gy; 1])    K=11 (21 mixed)
  L2:  y2 = relu(blockdiag(w1) @ y1) + b1        K=64/56 split tiles,
       bias applied by the PSUM-evacuation op (ACT bias / DVE add+max)
  L3:  out = blockdiag(w2) @ y2  + b2            K=128, M=15 col-tiled,
       b2 applied by the stage-copy op

Perf structure:
  - whole matmul path fp16 (1 cycle/row, rel-err ~1e-3)
  - L1 groups on distinct 32-row PE row-groups; L2 split into 64x64-ish
    quadrant tiles with alternating group parity so 4 tiles run
    concurrently; L3 on distinct 32-col groups
  - PSUM pair tiles [128,1024]: two groups share one tile; wide evac ops
  - controller DMAs ride sync+scalar alone; features ride gpsimd;
    SBUF memsets ride gpsimd; dummy matmuls bridge the staging gap
"""

import numpy as np

import concourse.bass as bass
import concourse.bacc as bacc
import concourse.tile as tile
from concourse import mybir
from concourse import bass_utils

F32 = mybir.dt.float32
F16 = mybir.dt.float16

HID = 256
NP = 169
Q = 50
H, W = 72, 120
P = H * W            # 8640
NQ = 100
NCORES = 8
STRIDE = 4

CHUNK = 512
CHUNKS = [(i * CHUNK, CHUNK) for i in range(16)] + [(16 * CHUNK, P - 16 * CHUNK)]
MMN = 512

# group table: (kind, qbase, nq, band) ; kind 0=f0, 1=f1, 2=mixed
# Ft bands: 0 = f0, 32 = f1, 64 = mixed [f0;-gx;-gy;1;f1;-gx;-gy], 96 = f1.
GROUPS = [
    (0, 0, 15, 0),      # g0 f0 q0-14
    (0, 15, 15, 64),    # g1 f0 q15-29 (head rows of the mixed band)
    (0, 30, 15, 0),     # g2 f0 q30-44
    (1, 0, 15, 32),     # g3 f1 q0-14
    (1, 15, 15, 96),    # g4 f1 q15-29
    (1, 30, 15, 32),    # g5 f1 q30-44
    (2, 45, 10, 64),    # g6 mixed q45-49 both frames (K=21, M=104)
]
# L2 quadrant parity: even groups y2 = [q0-7 @0-63, q8-14 @64-119];
# odd groups swapped [q8-14 @0-55, q0-7 @64-127]. g6 even, halves 5q+5q.
PARITY = [0, 1, 0, 0, 1, 1, 0]

ACT_OVH, ACT_RATE = 352.0 / 1.2, 1.0 / 1.2
DVE_OVH, DVE_RATE = 150.0, 1.0 / 0.96


def _build_program():
    nc = bacc.Bacc("TRN2", target_bir_lowering=False, debug=False)

    mf = nc.dram_tensor("mf", [2, 8, P], F16, kind="ExternalInput").ap()
    hsz = nc.dram_tensor("hsz", [HID + 1, NQ], F16, kind="ExternalInput").ap()
    wdev = nc.dram_tensor("wdev", [HID + 1, NP], F16, kind="ExternalInput").ap()
    refs = nc.dram_tensor("refs", [2, 1024], F32, kind="ExternalInput").ap()
    cst = nc.dram_tensor("cst", [3, P], F16, kind="ExternalInput").ap()
    zz = nc.dram_tensor("zz", [128, 1024], F16, kind="ExternalInput").ap()
    outp = nc.dram_tensor("outp", [NQ, P], F32, kind="ExternalOutput").ap()

    Relu = mybir.ActivationFunctionType.Relu
    Copy = mybir.ActivationFunctionType.Copy
    Ident = mybir.ActivationFunctionType.Identity
    ADD = mybir.AluOpType.add
    MAX = mybir.AluOpType.max

    eng_t = {"act": 0.0, "dve": 5000.0}

    def pick_engine(fd):
        ca = eng_t["act"] + ACT_OVH + fd * ACT_RATE
        cd = eng_t["dve"] + DVE_OVH + fd * DVE_RATE
        if ca <= cd:
            eng_t["act"] = ca
            return "act"
        eng_t["dve"] = cd
        return "dve"

    def relu_to(out_ap, in_ap, fd):
        if pick_engine(fd) == "act":
            nc.scalar.activation(out_ap, in_ap, Relu)
        else:
            nc.vector.tensor_scalar(out=out_ap, in0=in_ap, scalar1=0.0,
                                    scalar2=None, op0=MAX)

    def relu_bias_to(out_ap, in_ap, bias_ap, fd):
        if pick_engine(fd) == "act":
            nc.scalar.activation(out_ap, in_ap, Relu, bias=bias_ap)
        else:
            nc.vector.tensor_scalar(out=out_ap, in0=in_ap, scalar1=bias_ap,
                                    scalar2=0.0, op0=ADD, op1=MAX)

    def add_bias_to(out_ap, in_ap, bias_ap, fd):
        if pick_engine(fd) == "act":
            nc.scalar.activation(out_ap, in_ap, Ident, bias=bias_ap)
        else:
            nc.vector.tensor_scalar(out=out_ap, in0=in_ap, scalar1=bias_ap,
                                    scalar2=None, op0=ADD)

    with tile.TileContext(nc) as tc:
        with tc.tile_pool(name="persist", bufs=1) as pers:

            # ---------------- persistent SBUF ----------------
            Ft = pers.tile([128, P], F16, tag="F")
            wsb1 = pers.tile([128, NP], F16, tag="wsb1")
            wsb2 = pers.tile([128, NP], F16, tag="wsb2")
            wsb3 = pers.tile([1, NP], F16, tag="wsb3")
            hsb1 = pers.tile([128, NQ], F16, tag="hsb1")
            hsb2 = pers.tile([128, NQ], F16, tag="hsb2")
            hsb3 = pers.tile([1, NQ], F16, tag="hsb3")
            refsb = pers.tile([2, 1024], F32, tag="refsb")
            PW0S = pers.tile([11, 1024], F32, tag="PW0S")
            PW1S = pers.tile([9, 1024], F32, tag="PW1S")
            PW23S = pers.tile([9, NQ], F32, tag="PW23S")
            ctile = pers.tile([1, 1024], F32, tag="ctile")
            vtmp = pers.tile([3, 1024], F32, tag="vtmp")
            w89 = pers.tile([2, 1024], F32, tag="w89")
            ones3 = pers.tile([3, 1], F32, tag="ones3")
            Sw = pers.tile([10, NQ * 8], F16, tag="Sw")
            Sc = pers.tile([1, NQ * 8], F16, tag="Sc")
            Sb1 = pers.tile([1, NQ * 8], F16, tag="Sb1")
            b1raw = pers.tile([1, 1024], F32, tag="b1raw")
            W1C2 = pers.tile([8, 840], F16, tag="W1C2")
            W23G = pers.tile([9, 105], F16, tag="W23G")
            b1v = [pers.tile([128, 1], F32, name=f"b1v_{g}", tag=f"b1v_{g}")
                   for g in range(7)]
            b2v = [pers.tile([128, 1], F32, name=f"b2v_{p}", tag=f"b2v_{p}")
                   for p in range(2)]

            T1 = pers.tile([128, 840], F16, tag="T1")
            T2 = pers.tile([120, 840], F16, tag="T2")
            T3 = pers.tile([128, 112], F16, tag="T3")
            y1l = [pers.tile([120, 1024], F16, name=f"y1_{i}", tag=f"y1_{i}")
                   for i in range(4)]
            y2l = [pers.tile([128, 1024], F16, name=f"y2_{i}", tag=f"y2_{i}")
                   for i in range(4)]
            stageA = pers.tile([128, P], F32, tag="stageA")
            stageB = pers.tile([128, P], F32, tag="stageB")

            # ---------------- input DMAs ----------------
            # sync+scalar carry ONLY the controller inputs first, so the
            # controller matmuls can start ~3us in; features ride gpsimd.
            nc.sync.dma_start(out=wsb1[:, :], in_=wdev[0:128, :])
            nc.scalar.dma_start(out=hsb1[:, :], in_=hsz[0:128, :])
            nc.sync.dma_start(out=wsb2[:, :], in_=wdev[128:256, :])
            nc.scalar.dma_start(out=hsb2[:, :], in_=hsz[128:256, :])
            nc.sync.dma_start(out=wsb3[:, :], in_=wdev[256:257, :])
            nc.scalar.dma_start(out=hsb3[:, :], in_=hsz[256:257, :])
            nc.sync.dma_start(out=refsb[:, :], in_=refs)

            for base, fr in ((0, 0), (32, 1), (96, 1)):
                nc.gpsimd.dma_start(out=Ft[base:base + 8, :], in_=mf[fr])
                nc.gpsimd.dma_start(out=Ft[base + 8:base + 11, :], in_=cst[0:3, :])
            nc.gpsimd.dma_start(out=Ft[64:72, :], in_=mf[0])
            nc.gpsimd.dma_start(out=Ft[72:75, :], in_=cst[0:3, :])
            nc.gpsimd.dma_start(out=Ft[75:83, :], in_=mf[1])
            nc.gpsimd.dma_start(out=Ft[83:85, :], in_=cst[0:2, :])

            # ---- zero fills via DRAM-zeros DMAs on the build queues ----
            # (same queues as the dependent T-build DMAs -> natural FIFO
            # ordering, no cross-queue stalls; gpsimd handles bias vecs)
            nc.sync.dma_start(out=T2[0:120, :], in_=zz[0:120, 0:840])
            nc.scalar.dma_start(out=T3[:, :], in_=zz[0:128, 0:112])
            nc.sync.dma_start(out=T1[64:85, 720:800], in_=zz[0:21, 0:80])
            for t in b1v + b2v:
                nc.gpsimd.memset(t[:, :], 0.0)

            # ------------- controller matmuls (fp16, dense) --------------
            with tc.tile_pool(name="psctrl", bufs=1, space="PSUM") as psc:
                pw0 = psc.tile([11, 1024], F32, tag="pw0")
                pw1 = psc.tile([9, 1024], F32, tag="pw1")
                pw23 = psc.tile([9, NQ], F32, tag="pw23")
                kchunks = [(wsb1, hsb1, 128), (wsb2, hsb2, 128), (wsb3, hsb3, 1)]
                for o in range(8):
                    for kc, (wk, hk, kn) in enumerate(kchunks):
                        nc.tensor.matmul(
                            pw0[0:11, o * 128:o * 128 + NQ],
                            wk[0:kn, o * 11:o * 11 + 11], hk[0:kn, :],
                            start=(kc == 0), stop=(kc == 2))
                for o2 in range(8):
                    for kc, (wk, hk, kn) in enumerate(kchunks):
                        nc.tensor.matmul(
                            pw1[0:9, o2 * 128:o2 * 128 + NQ],
                            wk[0:kn, 88 + o2 * 9:88 + o2 * 9 + 9], hk[0:kn, :],
                            start=(kc == 0), stop=(kc == 2))
                for kc, (wk, hk, kn) in enumerate(kchunks):
                    nc.tensor.matmul(
                        pw23[0:9, 0:NQ],
                        wk[0:kn, 160:169], hk[0:kn, :],
                        start=(kc == 0), stop=(kc == 2))

                nc.vector.tensor_copy(PW0S[:, :], pw0[:, :])
                nc.scalar.activation(PW1S[:, :], pw1[:, :], Copy)
                nc.scalar.activation(PW23S[:, :], pw23[:, :], Copy)

                # c row: c[o*128+n] = w8*refx + w9*refy + b0
                nc.sync.dma_start(out=w89[0:2, :], in_=PW0S[8:10, :])
                nc.vector.tensor_tensor(out=vtmp[0:2, :], in0=w89[0:2, :],
                                        in1=refsb[0:2, :],
                                        op=mybir.AluOpType.mult)
                nc.sync.dma_start(out=vtmp[2:3, :], in_=PW0S[10:11, :])
                nc.vector.memset(ones3[:, :], 1.0)
                pc = psc.tile([1, 1024], F32, tag="pc")
                nc.tensor.matmul(pc[0:1, 0:512], ones3[0:3, 0:1],
                                 vtmp[0:3, 0:512], start=True, stop=True)
                nc.tensor.matmul(pc[0:1, 512:1024], ones3[0:3, 0:1],
                                 vtmp[0:3, 512:1024], start=True, stop=True)
                nc.vector.tensor_copy(ctile[0:1, :], pc[0:1, :])

                # dummy matmuls: keep the PE array busy while staging runs
                pwm = psc.tile([128, MMN], F32, tag="pwm")
                for wi in range(18):
                    nc.tensor.matmul(
                        pwm[0:100, 0:NP],
                        hsb1[0:128, 0:NQ], wsb1[0:128, 0:NP],
                        start=True, stop=True)

            # ------- interleaved staging (fp32 -> fp16) -------
            w0r = PW0S.rearrange("p (o n) -> p n o", o=8)
            cr = ctile.rearrange("p (o n) -> p n o", o=8)
            w1r = PW1S.rearrange("p (o n) -> p n o", o=8)
            nc.vector.tensor_copy(Sw[0:10, :], w0r[0:10, 0:NQ, :])
            nc.vector.tensor_copy(Sc[0:1, :], cr[0:1, 0:NQ, :])
            nc.sync.dma_start(out=b1raw[0:1, :], in_=PW1S[8:9, :])
            b1r = b1raw.rearrange("p (o n) -> p n o", o=8)
            nc.vector.tensor_copy(Sb1[0:1, :], b1r[0:1, 0:NQ, :])
            for dst0, n0, n1 in ((0, 0, 45), (360, 50, 95),
                                 (720, 45, 50), (760, 95, 100)):
                nc.vector.tensor_copy(W1C2[0:8, dst0:dst0 + (n1 - n0) * 8],
                                      w1r[0:8, n0:n1, :])
            # W23G in TBLOCK order: g0,g2,g3 | g6(f0,f1) | g1,g4,g5
            for dst0, n0, n1 in ((0, 0, 15), (15, 30, 45), (30, 50, 65),
                                 (45, 45, 50), (50, 95, 100),
                                 (60, 15, 30), (75, 65, 80), (90, 80, 95)):
                nc.scalar.activation(W23G[0:9, dst0:dst0 + n1 - n0],
                                     PW23S[0:9, n0:n1], Copy)

            # ------------- per-group weight builds (fp16 block DMAs) -----
            _brot = [[nc.sync, nc.scalar], [nc.gpsimd]]
            _bi = [0]
            _bphase = [0]

            def bdma(**kw):
                rot = _brot[_bphase[0]]
                e = rot[_bi[0] % len(rot)]
                _bi[0] += 1
                e.dma_start(**kw)

            P1 = PW1S.ap[0][0]
            P23 = PW23S.ap[0][0]
            P2t = T2.ap[0][0]
            P3t = T3.ap[0][0]
            PWc = W1C2.ap[0][0]
            PWg = W23G.ap[0][0]

            def t1_build(g):
                kind, qb, nq, band = GROUPS[g]
                n0 = (50 * (1 if kind == 1 else 0) + qb) * 8
                c0 = g * 120
                m = nq * 8
                if kind < 2:
                    bdma(out=T1[band:band + 10, c0:c0 + m],
                         in_=Sw[0:10, n0:n0 + m])
                    bdma(out=T1[band + 10:band + 11, c0:c0 + m],
                         in_=Sc[0:1, n0:n0 + m])
                else:  # mixed: f0 q45-49 at cols 0-39, f1 q45-49 at 40-79
                    bdma(out=T1[64:74, c0:c0 + 40], in_=Sw[0:10, 360:400])
                    bdma(out=T1[74:75, c0:c0 + 40], in_=Sc[0:1, 360:400])
                    bdma(out=T1[75:85, c0 + 40:c0 + 80], in_=Sw[0:10, 760:800])
                    bdma(out=T1[74:75, c0 + 40:c0 + 80], in_=Sc[0:1, 760:800])

            SbP = Sb1.ap[0][0]

            def b1v_build(g):
                kind, qb, nq, band = GROUPS[g]
                bP = b1v[g].ap[0][0]
                if kind == 2:  # g6 un-split: rows 0-79 = f0 q45-49, f1 q45-49
                    specs = [(0, 45, 5), (40 * bP, 95, 5)]
                elif PARITY[g]:
                    n0 = 50 * kind + qb
                    specs = [(64 * bP, n0, 8), (0, n0 + 8, 7)]
                else:
                    n0 = 50 * kind + qb
                    specs = [(0, n0, 15)]
                for doff, nsrc, cnt in specs:
                    src = bass.AP(tensor=Sb1.tensor,
                                  offset=Sb1.offset + nsrc * 8,
                                  ap=[[SbP, 1], [1, cnt * 8]])
                    dst = bass.AP(tensor=b1v[g].tensor,
                                  offset=b1v[g].offset + doff,
                                  ap=[[bP, cnt * 8], [1, 1]])
                    nc.gpsimd.dma_start(out=dst, in_=src)  # casting DMA

            # T3 column-block order: even groups first so each j-DMA can
            # stride uniformly over same-row-layout groups.
            # blocks: g0,g2,g3 (even 15q) | g6 | g1,g4,g5 (odd 15q)
            TBLOCK = {0: 0, 2: 1, 3: 2, 6: 3, 1: 4, 4: 5, 5: 6}

            def t3_build_even():
                # groups g0,g2,g3,g6 (blocks 0-3): rows j*8, uniform strides
                for j in range(15):
                    gcnt = 4 if j < 10 else 3
                    dst = bass.AP(tensor=T3.tensor,
                                  offset=T3.offset + j * 8 * P3t + j,
                                  ap=[[P3t, 8], [16, gcnt], [1, 1]])
                    src = bass.AP(tensor=W23G.tensor,
                                  offset=W23G.offset + j,
                                  ap=[[PWg, 8], [15, gcnt], [1, 1]])
                    bdma(out=dst, in_=src)

            def t3_build_odd():
                for j in range(15):
                    row = 64 + j * 8 if j < 8 else (j - 8) * 8
                    dst = bass.AP(tensor=T3.tensor,
                                  offset=T3.offset + row * P3t + 64 + j,
                                  ap=[[P3t, 8], [16, 3], [1, 1]])
                    src = bass.AP(tensor=W23G.tensor,
                                  offset=W23G.offset + 60 + j,
                                  ap=[[PWg, 8], [15, 3], [1, 1]])
                    bdma(out=dst, in_=src)

            def b2v_build(pi, glist):
                # b2 values sit on W23G row 8 in TBLOCK column order
                bP = b2v[pi].ap[0][0]
                for gi, g in enumerate(glist):
                    nq = GROUPS[g][2]
                    src = bass.AP(tensor=W23G.tensor,
                                  offset=W23G.offset + 8 * PWg + TBLOCK[g] * 15,
                                  ap=[[PWg, 1], [1, nq]])
                    dst = bass.AP(tensor=b2v[pi].tensor,
                                  offset=b2v[pi].offset + 32 * gi * bP,
                                  ap=[[bP, nq], [1, 1]])
                    nc.gpsimd.dma_start(out=dst, in_=src)  # casting DMA

            # phase-A builds
            for g in (0, 1, 3, 4):
                t1_build(g)
                b1v_build(g)
            b2v_build(0, [0, 1, 3, 4])
            # T2 block-diagonal: one DMA per j covering all 7 groups
            for j in range(15):
                gcnt = 7 if j < 10 else 6
                src = bass.AP(tensor=W1C2.tensor, offset=W1C2.offset + j * 8,
                              ap=[[PWc, 8], [120, gcnt], [1, 8]])
                dst = bass.AP(tensor=T2.tensor,
                              offset=T2.offset + j * 8 * P2t + j * 8,
                              ap=[[P2t, 8], [120, gcnt], [1, 8]])
                bdma(out=dst, in_=src)
            t3_build_even()
            t3_build_odd()
            # ---- phase-B-only builds ----
            _bphase[0] = 1
            for g in (2, 5, 6):
                t1_build(g)
                b1v_build(g)
            b2v_build(1, [2, 5, 6])

            # ---------------- main loop: two phases ----------------
            psm_cm = tc.tile_pool(name="psmain", bufs=2, space="PSUM")
            psm = psm_cm.__enter__()
            QUARTERS = [(0, 4), (4, 8), (8, 12), (12, 17)]
            # strip format: (src_row, n_strips, rows_per_strip, out_row)
            # consecutive strips sit 32 partitions apart and write
            # consecutive output-row blocks -> one 3D DMA when n_strips>1
            PHASES = [
                ([(0, 1), (3, 4)], stageA,
                 [(0, 2, 15, 0), (64, 2, 15, 50)]),
                ([(2, 5), (6,)], stageB,
                 [(0, 1, 15, 30), (32, 1, 15, 80), (64, 1, 5, 45),
                  (69, 1, 5, 95)]),
            ]
            out_eng = [nc.sync, nc.gpsimd]
            for pi, (pairs, stage_t, strips) in enumerate(PHASES):
                glist = [g for pr in pairs for g in pr]
                glen = len(glist)
                for ci, (coff, clen) in enumerate(CHUNKS):
                    # ---- L1 ----
                    ps1s = []
                    for wi, pr in enumerate(pairs):
                        ps1 = psm.tile([128, 1024], F32, tag="ps1", bufs=2,
                                       name=f"ps1_{pi}_{ci}_{wi}")
                        for w, g in enumerate(pr):
                            kind, qb, nq, band = GROUPS[g]
                            m = nq * 8
                            k1 = 21 if kind == 2 else 11
                            nc.tensor.matmul(
                                ps1[0:m, w * 512:w * 512 + clen],
                                T1[band:band + k1, g * 120:g * 120 + m],
                                Ft[band:band + k1, coff:coff + clen],
                                start=True, stop=True,
                                tile_position=(band, 0))
                        ps1s.append(ps1)
                    # ---- relu1 (wide, no bias) ----
                    y1s = []
                    for wi, pr in enumerate(pairs):
                        y1 = y1l[(ci * 2 + wi) % 4]
                        if len(pr) == 2:
                            fd, mrow = 512 + clen, 120
                        else:
                            fd, mrow = clen, 80
                        relu_to(y1[0:mrow, 0:fd], ps1s[wi][0:mrow, 0:fd], fd)
                        y1s.append(y1)
                    # ---- L2 (quadrant tiles, 4-way concurrent) ----
                    ps2s = []
                    for wi, pr in enumerate(pairs):
                        ps2 = psm.tile([128, 1024], F32, tag="ps2", bufs=2,
                                       name=f"ps2_{pi}_{ci}_{wi}")
                        if ci == 0:
                            nc.vector.memset(ps2[:, :], 0.0)
                        for w, g in enumerate(pr):
                            c0 = g * 120
                            ws = w * 512
                            y1 = y1s[wi]
                            if GROUPS[g][0] == 2:      # g6: single K=80 tile
                                nc.tensor.matmul(
                                    ps2[0:80, ws:ws + clen],
                                    T2[0:80, c0:c0 + 80],
                                    y1[0:80, ws:ws + clen],
                                    start=True, stop=True,
                                    tile_position=(0, 0))
                            elif not PARITY[g]:
                                nc.tensor.matmul(
                                    ps2[0:64, ws:ws + clen],
                                    T2[0:64, c0:c0 + 64],
                                    y1[0:64, ws:ws + clen],
                                    start=True, stop=True,
                                    tile_position=(0, 0))
                                nc.tensor.matmul(
                                    ps2[64:120, ws:ws + clen],
                                    T2[64:120, c0 + 64:c0 + 120],
                                    y1[64:120, ws:ws + clen],
                                    start=True, stop=True,
                                    tile_position=(64, 64))
                            else:
                                nc.tensor.matmul(
                                    ps2[64:128, ws:ws + clen],
                                    T2[0:64, c0:c0 + 64],
                                    y1[0:64, ws:ws + clen],
                                    start=True, stop=True,
                                    tile_position=(0, 64))
                                nc.tensor.matmul(
                                    ps2[0:56, ws:ws + clen],
                                    T2[64:120, c0 + 64:c0 + 120],
                                    y1[64:120, ws:ws + clen],
                                    start=True, stop=True,
                                    tile_position=(64, 0))
                        ps2s.append(ps2)
                    # ---- relu2 + bias (per group window) ----
                    y2s = []
                    for wi, pr in enumerate(pairs):
                        y2 = y2l[(ci * 2 + wi) % 4]
                        for w, g in enumerate(pr):
                            ws = w * 512
                            if GROUPS[g][0] == 2:
                                mrow = 80
                            elif PARITY[g]:
                                mrow = 128
                            else:
                                mrow = 120
                            relu_bias_to(y2[0:mrow, ws:ws + clen],
                                         ps2s[wi][0:mrow, ws:ws + clen],
                                         b1v[g][0:mrow, 0:1], clen)
                        y2s.append(y2)
                    # ---- L3 (col-tiled, K=128) ----
                    mtop = 32 * (glen - 1) + GROUPS[glist[-1]][2]
                    ph = psm.tile([128, 1024], F32, tag="ps2", bufs=2,
                                  name=f"ps3_{pi}_{ci}")
                    gi = 0
                    for wi, pr in enumerate(pairs):
                        for w, g in enumerate(pr):
                            nqg = GROUPS[g][2]
                            s = 32 * gi
                            gi += 1
                            tb = TBLOCK[g] * 16
                            if GROUPS[g][0] == 2:
                                k3 = 80
                            elif PARITY[g]:
                                k3 = 128
                            else:
                                k3 = 120
                            nc.tensor.matmul(
                                ph[s:s + nqg, 0:clen],
                                T3[0:k3, tb:tb + nqg],
                                y2s[wi][0:k3, w * 512:w * 512 + clen],
                                start=True, stop=True,
                                tile_position=(0, s),
                                skip_group_check=True)
                    add_bias_to(stage_t[0:mtop, coff:coff + clen],
                                ph[0:mtop, 0:clen], b2v[pi][0:mtop, 0:1], clen)

                    for qi, (c0i, c1i) in enumerate(QUARTERS):
                        if ci == c1i - 1:
                            p0 = CHUNKS[c0i][0]
                            p1 = coff + clen
                            Pst = stage_t.ap[0][0]
                            for si, (srow, cnt, nrow, orow) in enumerate(strips):
                                if cnt == 1:
                                    out_eng[si % 2].dma_start(
                                        out=outp[orow:orow + nrow, p0:p1],
                                        in_=stage_t[srow:srow + nrow, p0:p1])
                                else:
                                    sap = bass.AP(
                                        tensor=stage_t.tensor,
                                        offset=stage_t.offset + srow * Pst + p0,
                                        ap=[[32 * Pst, cnt], [Pst, nrow],
                                            [1, p1 - p0]])
                                    dap = bass.AP(
                                        tensor=outp.tensor,
                                        offset=outp.offset + orow * P + p0,
                                        ap=[[nrow * P, cnt], [P, nrow],
                                            [1, p1 - p0]])
                                    out_eng[si % 2].dma_start(out=dap, in_=sap)
            psm_cm.__exit__(None, None, None)

    nc.compile()
    return nc


_NC = None


def _get_nc():
    global _NC
    if _NC is None:
        _NC = _build_program()
    return _NC


def _host_pack(hs, mask_features, references, sizes, W_ctrl, b_ctrl):
    hs = np.asarray(hs, np.float32)
    mask_features = np.asarray(mask_features, np.float32)
    references = np.asarray(references, np.float32)
    sizes = np.asarray(sizes, np.float32)
    W_ctrl = np.asarray(W_ctrl, np.float32)
    b_ctrl = np.asarray(b_ctrl, np.float32)

    xs = np.arange(W, dtype=np.float32) * STRIDE + STRIDE // 2
    ys = np.arange(H, dtype=np.float32) * STRIDE + STRIDE // 2
    gxf = np.tile(xs, H)
    gyf = np.repeat(ys, W)
    cstm = np.stack([-gxf, -gyf, np.ones(P, np.float32)]).astype(np.float16)

    W_aug = np.concatenate([W_ctrl.T, b_ctrl[None, :]], 0)  # [257, 169]
    perm = []
    for o in range(8):
        perm += [o * 10 + i for i in range(10)] + [152 + o]
    for o2 in range(8):
        perm += [80 + o2 * 8 + oo for oo in range(8)] + [160 + o2]
    perm += [144 + oo for oo in range(8)] + [168]
    wdev = np.ascontiguousarray(W_aug[:, perm]).astype(np.float16)

    b_idx = np.arange(16) // 8
    scale = sizes[b_idx][:, ::-1]
    refs_px = references * scale[:, None, :]

    in_maps = []
    for c in range(NCORES):
        hs_c = hs[2 * c:2 * c + 2].reshape(NQ, HID)
        hsz = np.concatenate([hs_c.T, np.ones((1, NQ), np.float32)],
                             0).astype(np.float16)
        mf_c = mask_features[2 * c:2 * c + 2].reshape(2, 8, P).astype(np.float16)
        rp = refs_px[2 * c:2 * c + 2].reshape(NQ, 2)
        refs_rep = np.zeros((2, 1024), np.float32)
        for o in range(8):
            refs_rep[0, o * 128:o * 128 + NQ] = rp[:, 0]
            refs_rep[1, o * 128:o * 128 + NQ] = rp[:, 1]
        in_maps.append(dict(
            mf=np.ascontiguousarray(mf_c),
            hsz=np.ascontiguousarray(hsz),
            wdev=wdev,
            refs=refs_rep,
            cst=cstm,
            zz=np.zeros((128, 1024), np.float16),
        ))
    return in_maps


def kernel(hs, mask_features, references, sizes, W_ctrl, b_ctrl, T):
    assert int(T) == 8
    nc = _get_nc()
    in_maps = _host_pack(hs, mask_features, references, sizes, W_ctrl, b_ctrl)
    res = bass_utils.run_bass_kernel_spmd(nc, in_maps, core_ids=list(range(NCORES)))
    out = np.empty((16, Q, H, W), np.float32)
    for c in range(NCORES):
        out[2 * c:2 * c + 2] = res.results[c]["outp"].reshape(2, Q, H, W)
    return out
